# revision 68
# baseline (speedup 1.0000x reference)
"""Trainium2 Bass kernel for AttentionWithRotaryPositionalEmbedding.

Problem shapes (hardcoded): x [4, 2048, 512], 8 heads, head dim 64.
Sharding: 8 cores = (batch b = core//2) x (query half = core%2).
Each core computes a [1024, 512] slice of the output; k/v are computed
locally from the full x[b] so no collectives are needed.

Key perf insight (measured on hw): the PE runs matmuls at 2.4 GHz only
while it has a continuous backlog of ready work; whenever it idles
waiting on semaphores (e.g. softmax EXP on the ACT engine pacing the
attention loop), its clock drops to 1.2 GHz and every matmul takes 2x.
So this kernel emits ONE globally software-pipelined schedule: the
scores->EXP->attn@v chunk pipeline is interleaved with "filler" work
(qkv projection chains, RoPE rotation matmuls, v-projection chains,
per-head softmax-denominator matmuls) consumed greedily whenever the
estimated PE timeline falls behind the estimated ACT timeline.

Attention math per core (fp16 operands, f32 psum accumulation):
  scores sT[k,q] = kT_h^T qT_h per 128-key chunk; EXP on ACT with fused
  *0.125; attn@v with lhsT = [v_h | exp(mask)] (M=65) accumulated over
  16 chunks (psum row 64 = softmax denominators); per-head denominator
  replication via K=1 ones matmul + fast reciprocal + normalize; output
  projection as a dense tail of accumulation chains with bias folded in
  as a K=1 matmul.
"""

import sys

import numpy as np

if "/opt/trn_rl_repo" not in sys.path:
    sys.path.insert(0, "/opt/trn_rl_repo")

B, N, C = 4, 2048, 512
H, DH = 8, 64
NQ = 1024  # queries per core
P = 128
NCHUNK = N // P  # 16 k chunks
VW = DH + 1  # v columns incl. the emask/ones column
MAX_FPS = np.float32(30.0)

_CACHE = {}


def _host_prep(x, mask, times, Wqkv, Wproj, bproj):
    """Build per-core input maps (numpy only)."""
    x = np.asarray(x, np.float32)
    mask = np.asarray(mask, np.float32)
    times = np.asarray(times, np.float32)
    Wqkv = np.asarray(Wqkv, np.float32)
    Wproj = np.asarray(Wproj, np.float32)
    bproj = np.asarray(bproj, np.float32).reshape(1, C)

    wt = np.ascontiguousarray(Wqkv.T).astype(np.float16)  # [512,1536]=[WqT|WkT|WvT]
    wpt = np.ascontiguousarray(Wproj.T).astype(np.float16)  # [512, 512]

    # pairwise rotation permutation: (M2 @ v)[2i] = -v[2i+1]; [2i+1] = +v[2i]
    M2 = np.zeros((P, P), np.float16)
    for i in range(P // 2):
        M2[2 * i, 2 * i + 1] = -1.0
        M2[2 * i + 1, 2 * i] = 1.0
    m2t = np.ascontiguousarray(M2.T)

    # rotary tables (computed f32 on host, stored fp16 on device)
    inv_freq = (np.float32(1.0) /
                (np.float32(10000.0) **
                 (np.arange(0, DH, 2, dtype=np.float32) / np.float32(DH))))  # [32]
    pos = np.round(times * MAX_FPS)  # [B, N] f32, round-half-even like jnp

    in_maps = []
    for core in range(8):
        b, qhalf = core // 2, core % 2
        if qhalf == 0:
            perm = np.arange(N)
        else:
            perm = np.r_[NQ:N, 0:NQ]
        xt = np.ascontiguousarray(x[b].T[:, perm]).astype(np.float16)  # [512, 2048]
        freqs = pos[b][perm][None, :] * inv_freq[:, None]     # [32, 2048] f32
        cos32 = np.cos(freqs.astype(np.float32))
        sin32 = np.sin(freqs.astype(np.float32))
        ridx = (np.arange(P) % DH) // 2                       # row -> pair index
        ce = np.concatenate([cos32[ridx], sin32[ridx]], axis=1)  # [128, 4096]
        ce = np.ascontiguousarray(ce.astype(np.float16))
        em = np.exp(mask[b][perm]).astype(np.float32)         # [2048]
        emask = np.ascontiguousarray(em.reshape(NCHUNK, P).T)  # [128, 16]
        # chalf (fp16): m2t 0:128 | ones 128:256 | bias row 256:768
        chalf = np.zeros((P, 768), np.float16)
        chalf[:, 0:128] = m2t
        chalf[:, 128:256] = 1.0
        chalf[0, 256:768] = bproj[0].astype(np.float16)
        in_maps.append({
            "xt": xt, "wt": wt, "wpt": wpt,
            "ce": ce, "chalf": chalf, "emask": emask,
        })
    return in_maps


def _build_module():
    import concourse.tile as tile
    import concourse.mybir as mybir
    from concourse import bacc

    f32 = mybir.dt.float32
    f32r = mybir.dt.float32r
    f16 = mybir.dt.float16
    f8 = mybir.dt.float8e4
    DROW = mybir.MatmulPerfMode.DoubleRow
    nc = bacc.Bacc(None, target_bir_lowering=False, debug=False)

    xt_d = nc.dram_tensor("xt", [C, N], f16, kind="ExternalInput")
    wt_d = nc.dram_tensor("wt", [C, 3 * C], f16, kind="ExternalInput")
    wpt_d = nc.dram_tensor("wpt", [C, C], f16, kind="ExternalInput")
    ce_d = nc.dram_tensor("ce", [P, 2 * N], f16, kind="ExternalInput")
    chalf_d = nc.dram_tensor("chalf", [P, 768], f16, kind="ExternalInput")
    emask_d = nc.dram_tensor("emask", [P, NCHUNK], f32, kind="ExternalInput")
    y_d = nc.dram_tensor("y", [NQ, C], f32, kind="ExternalOutput")

    EXPF = mybir.ActivationFunctionType.Exp
    COPYF = mybir.ActivationFunctionType.Copy
    MM = nc.tensor.matmul

    # virtual-time estimates (us) used only for schedule pacing
    MMT = 0.215
    EXPT = 0.95  # deliberately under actual (~1.1): overshoot the PE feed so
    # it never stalls (a stalled PE drops to 1.2 GHz and stays there)

    with tile.TileContext(nc) as tc:
        with (
            tc.tile_pool(name="consts", bufs=1) as consts,
            tc.tile_pool(name="big", bufs=1) as big,
            tc.tile_pool(name="expp", bufs=18) as expp,
            tc.tile_pool(name="prawp", bufs=3) as prawp,
            tc.tile_pool(name="tmps", bufs=3) as tmps,
            tc.tile_pool(name="rrp", bufs=2) as rrp,
            tc.tile_pool(name="ypool", bufs=2) as ypool,
            tc.tile_pool(name="pss", bufs=2, space="PSUM") as pss,
            tc.tile_pool(name="psav", bufs=1, space="PSUM") as psav,
            tc.tile_pool(name="pgen", bufs=2, space="PSUM") as pgen,
        ):
            # ---------------- persistent tiles ----------------
            blob = consts.tile([P, 768], f16, name="blob")
            emask_t = consts.tile([P, NCHUNK], f32, name="emask")
            # wpt packed as head PAIRS: rows 0:64 = even head dims, 64:128 odd
            wpt_s = consts.tile([P, (H // 2) * C], f16, name="wpt")
            wt_s = [big.tile([P, 3 * C], f16, name=f"wt{i}") for i in range(4)]
            xt_s = [big.tile([P, N], f16, name=f"xt{i}") for i in range(4)]
            ce_s = big.tile([P, 2 * N], f16, name="ce")
            qT = [big.tile([P, NQ], f16, name=f"qT{i}") for i in range(4)]
            kT = [big.tile([P, N], f16, name=f"kT{i}") for i in range(4)]
            # v: [tok, chunk, head, 64 v dims + emask col]
            v65 = big.tile([P, NCHUNK, H, VW], f16, name="v65")
            sc = [big.tile([VW, NQ], f16, name=f"sc{h}") for h in range(H)]
            # normalized sc packed as head pairs (via SBUF->SBUF DMA) so the
            # output projection runs full-square K=128 chains
            scp = [big.tile([P, NQ], f16, name=f"scp{i}") for i in range(H // 2)]

            m2t_s = blob[:, 0:128]
            ones_s = blob[:, 128:256]
            bias_s = blob[0:1, 256:768]
            emask_ap = emask_t[:, :]

            # ---------------- DMAs (priority order) ----------------
            # critical path first: wt q-ct0, ce piece0, xt nb0, wt k-ct0
            for ci in range(4):
                nc.sync.dma_start(wt_s[ci][:, 0:128], wt_d.ap()[ci * P:(ci + 1) * P, 0:128])
            nc.sync.dma_start(ce_s[:, 0:512], ce_d.ap()[:, 0:512])
            nc.sync.dma_start(ce_s[:, 2048:2560], ce_d.ap()[:, 2048:2560])
            for half in range(2):
                for ci in range(4):
                    nc.sync.dma_start(
                        xt_s[ci][:, half * 256:(half + 1) * 256],
                        xt_d.ap()[ci * P:(ci + 1) * P, half * 256:(half + 1) * 256])
            # second token block (q/k ct0 nb=1 gates head 0 too)
            for ci in range(4):
                nc.sync.dma_start(xt_s[ci][:, 512:1024],
                                  xt_d.ap()[ci * P:(ci + 1) * P, 512:1024])
            nc.sync.dma_start(ce_s[:, 512:1024], ce_d.ap()[:, 512:1024])
            nc.sync.dma_start(ce_s[:, 2560:3072], ce_d.ap()[:, 2560:3072])
            for ci in range(4):
                nc.sync.dma_start(wt_s[ci][:, 512:640], wt_d.ap()[ci * P:(ci + 1) * P, 512:640])
            nc.sync.dma_start(blob[:], chalf_d.ap())
            nc.sync.dma_start(emask_t[:], emask_d.ap())
            # v weights
            for ci in range(4):
                nc.sync.dma_start(wt_s[ci][:, 1024:1536], wt_d.ap()[ci * P:(ci + 1) * P, 1024:1536])
            # remaining xt
            for nb in range(2, 4):
                for ci in range(4):
                    nc.sync.dma_start(xt_s[ci][:, nb * 512:(nb + 1) * 512],
                                      xt_d.ap()[ci * P:(ci + 1) * P, nb * 512:(nb + 1) * 512])
            # remaining ce
            for piece in (slice(1024, 2048), slice(3072, 4096)):
                nc.sync.dma_start(ce_s[:, piece], ce_d.ap()[:, piece])
            # remaining wt (q ct1-3, k ct1-3)
            for ci in range(4):
                nc.sync.dma_start(wt_s[ci][:, 128:512], wt_d.ap()[ci * P:(ci + 1) * P, 128:512])
                nc.sync.dma_start(wt_s[ci][:, 640:1024], wt_d.ap()[ci * P:(ci + 1) * P, 640:1024])
            # output projection weights (needed only late)
            for hp in range(H // 2):
                nc.sync.dma_start(wpt_s[:, hp * C:(hp + 1) * C],
                                  wpt_d.ap()[hp * P:(hp + 1) * P, :])

            # ---------------- filler machinery ----------------
            st = {"pe": 0.0, "act": 0.0, "b": 0}  # b = bundles fully consumed
            rope_fifo = []  # pending proj groups: [push_idx, stage, sp, ct, nb, praw, tmp]

            def advance_rope():
                for item in list(rope_fifo):
                    age = st["b"] - item[0]
                    if item[1] == 0 and age >= 1:
                        sp, ct, nb, pr = item[2], item[3], item[4], item[5]
                        tok = nb * 512
                        t = tmps.tile([P, 2, 512], f16, name="tmp")
                        nc.vector.tensor_mul(t[:, 0, :], pr[:], ce_s[:, tok:tok + 512])
                        nc.vector.tensor_mul(t[:, 1, :], pr[:],
                                             ce_s[:, 2048 + tok:2048 + tok + 512])
                        item[1] = 1
                        item[6] = t
                    elif item[1] == 1 and age >= 2:
                        sp, ct, nb, t = item[2], item[3], item[4], item[6]
                        tok = nb * 512
                        pgm = pgen.tile([P, 512], f32, name="pg")
                        MM(pgm[:], m2t_s, t[:, 1, :], start=True, stop=True)
                        st["pe"] += MMT
                        dest = qT[ct] if sp == "q" else kT[ct]
                        nc.vector.tensor_add(dest[:, tok:tok + 512], t[:, 0, :], pgm[:])
                        rope_fifo.remove(item)

            # filler bundle order: ct0 first (gates head 0), v chunks early
            # (gate attn@v), later ct tiles spread through earlier heads.
            order = [("proj", "q", 0, 0), ("proj", "k", 0, 0),
                     ("proj", "q", 0, 1), ("proj", "k", 0, 1),
                     ("v", 0), ("proj", "k", 0, 2), ("v", 1),
                     ("proj", "k", 0, 3), ("v", 2), ("v", 3), ("v", 4),
                     ("proj", "q", 1, 0), ("v", 5), ("proj", "q", 1, 1),
                     ("v", 6), ("proj", "k", 1, 0), ("v", 7),
                     ("proj", "k", 1, 1), ("v", 8), ("proj", "k", 1, 2),
                     ("v", 9), ("proj", "k", 1, 3), ("v", 10), ("v", 11),
                     ("v", 12), ("v", 13),
                     ("proj", "q", 2, 0), ("v", 14), ("proj", "q", 2, 1),
                     ("v", 15),
                     ("proj", "k", 2, 0), ("proj", "k", 2, 1),
                     ("proj", "k", 2, 2), ("proj", "k", 2, 3),
                     ("proj", "q", 3, 0), ("proj", "q", 3, 1),
                     ("proj", "k", 3, 0), ("proj", "k", 3, 1),
                     ("proj", "k", 3, 2), ("proj", "k", 3, 3),
                     ("flush", 0), ("flush", 1)]
            pos = {it: i for i, it in enumerate(order)}

            # expand bundles into ~2-matmul filler units for fine interleave
            filler_units = []
            bctx = {}

            def mk_proj_units(sp, ct, nb):
                woff = 0 if sp == "q" else C
                key = (sp, ct, nb)

                def u1():
                    pg = pgen.tile([P, 512], f32, name="pg")
                    bctx[key] = pg
                    for ci in range(2):
                        MM(pg[:],
                           wt_s[ci][:, woff + ct * P: woff + (ct + 1) * P],
                           xt_s[ci][:, nb * 512:(nb + 1) * 512],
                           start=(ci == 0), stop=False)
                    st["pe"] += 2 * MMT

                def u2():
                    pg = bctx.pop(key)
                    for ci in range(2, 4):
                        MM(pg[:],
                           wt_s[ci][:, woff + ct * P: woff + (ct + 1) * P],
                           xt_s[ci][:, nb * 512:(nb + 1) * 512],
                           start=False, stop=(ci == 3))
                    st["pe"] += 2 * MMT
                    pr = prawp.tile([P, 512], f16, name="praw")
                    nc.vector.tensor_copy(pr[:], pg[:])
                    if ct == 0:
                        # preamble fast path: full RoPE chain inline (latency
                        # over throughput -- PE is DMA-stalled here anyway)
                        tok = nb * 512
                        t = tmps.tile([P, 2, 512], f16, name="tmp")
                        nc.vector.tensor_mul(t[:, 0, :], pr[:],
                                             ce_s[:, tok:tok + 512])
                        nc.vector.tensor_mul(t[:, 1, :], pr[:],
                                             ce_s[:, 2048 + tok:2048 + tok + 512])
                        pgm = pgen.tile([P, 512], f32, name="pg")
                        MM(pgm[:], m2t_s, t[:, 1, :], start=True, stop=True)
                        st["pe"] += MMT
                        dest = qT[ct] if sp == "q" else kT[ct]
                        nc.vector.tensor_add(dest[:, tok:tok + 512],
                                             t[:, 0, :], pgm[:])
                    else:
                        rope_fifo.append([st["b"], 0, sp, ct, nb, pr, None])
                    st["b"] += 1
                    advance_rope()
                return [u1, u2]

            def mk_v_units(c):
                key = ("v", c)

                def u1():
                    pg = pgen.tile([P, 512], f32, name="pg")
                    bctx[key] = pg
                    for ci in range(2):
                        MM(pg[:],
                           xt_s[ci][:, c * P:(c + 1) * P],
                           wt_s[ci][:, 2 * C:3 * C],
                           start=(ci == 0), stop=False)
                    st["pe"] += 2 * MMT

                def u2():
                    pg = bctx.pop(key)
                    for ci in range(2, 4):
                        MM(pg[:],
                           xt_s[ci][:, c * P:(c + 1) * P],
                           wt_s[ci][:, 2 * C:3 * C],
                           start=False, stop=(ci == 3))
                    st["pe"] += 2 * MMT
                    vv = v65[:, c, :, :]
                    # evacuate on ACT (idle during lead-in; DVE is the
                    # early-phase bottleneck)
                    nc.scalar.activation(
                        vv[:, :, 0:DH],
                        pg[:].rearrange("p (h w) -> p h w", w=DH),
                        COPYF, scale=emask_ap[:, c:c + 1])
                    st["act"] = max(st["act"], st["pe"]) + 0.78
                    nc.vector.tensor_copy(
                        vv[:, :, DH:DH + 1],
                        emask_ap[:, c:c + 1, None].to_broadcast((P, H, 1)))
                    st["b"] += 1
                    advance_rope()
                return [u1, u2]

            def mk_flush_unit():
                def u():
                    st["b"] += 1
                    advance_rope()
                return [u]

            for it in order:
                if it[0] == "proj":
                    filler_units.extend(mk_proj_units(it[1], it[2], it[3]))
                elif it[0] == "v":
                    filler_units.extend(mk_v_units(it[1]))
                else:
                    filler_units.extend(mk_flush_unit())

            def consume_unit():
                filler_units.pop(0)()

            def consume_until(idx):
                while st["b"] <= min(idx, len(order) - 1) and filler_units:
                    consume_unit()

            # bundle index that must be consumed before S(h, c) / A(h, c)
            def req_s(h, c):
                ct = h // 2
                lag = 0 if ct == 0 else 2  # ct0 ropes inline, no pipeline lag
                return max(pos[("proj", "q", ct, 0)], pos[("proj", "q", ct, 1)],
                           pos[("proj", "k", ct, c // 4)]) + lag
            def req_a(h, c):
                return pos[("v", c)] + 1

            # ---------------- attention pipeline ----------------
            LAG = 3
            STEPS = H * NCHUNK
            exs = {}
            pss_live = {}
            psos = {}
            post = []  # (due_step, fn) queue for per-head epilogue work

            def emit_s(h, c):
                kt, qt, pb = kT[h // 2], qT[h // 2], (h % 2) * DH
                ps = pss.tile([P, NQ], f32, name="ps_s")
                for qb in range(2):
                    MM(ps[:, qb * 512:(qb + 1) * 512],
                       kt[pb:pb + DH, c * P:(c + 1) * P],
                       qt[pb:pb + DH, qb * 512:(qb + 1) * 512],
                       start=True, stop=True)
                st["pe"] += 2 * MMT
                pss_live[(h, c)] = ps

            def emit_e(h, c):
                ps = pss_live.pop((h, c))
                ex = expp.tile([P, NQ], f16, name="ex")
                nc.scalar.activation(ex[:], ps[:], EXPF, scale=0.125)
                st["act"] = max(st["act"], st["pe"]) + EXPT
                exs[(h, c)] = ex

            def emit_a(h, c):
                if c == 0:
                    psos[h] = psav.tile([VW, NQ], f32, name="ps_o")
                ps_o = psos[h]
                ex = exs.pop((h, c))
                for qb in range(2):
                    MM(ps_o[:, qb * 512:(qb + 1) * 512],
                       v65[:, c, h, :],
                       ex[:, qb * 512:(qb + 1) * 512],
                       start=(c == 0), stop=(c == NCHUNK - 1))
                st["pe"] += 2 * MMT

            def mk_sccopy(h):
                def fn():
                    nc.vector.tensor_copy(sc[h][:], psos.pop(h)[:])
                return fn

            def mk_dennorm(h):
                def fn():
                    psd = pss.tile([P, NQ], f32, name="ps_s")
                    for qb in range(2):
                        MM(psd[0:DH, qb * 512:(qb + 1) * 512],
                           ones_s[DH:DH + 1, 0:DH],
                           sc[h][DH:DH + 1, qb * 512:(qb + 1) * 512],
                           start=True, stop=True)
                    st["pe"] += 2 * MMT
                    rr = rrp.tile([DH, NQ], f32, name="rr")
                    nc.vector.reciprocal_approx_fast(rr[:], psd[0:DH, :])
                    nc.vector.tensor_mul(sc[h][0:DH, :], sc[h][0:DH, :], rr[:])
                    for piece in range(2):
                        nc.sync.dma_start(
                            scp[h // 2][(h % 2) * DH:(h % 2) * DH + DH,
                                        piece * 512:(piece + 1) * 512],
                            sc[h][0:DH, piece * 512:(piece + 1) * 512])
                    if h == 5:
                        queue_op_partials()
                        op_start[0] = cur_s[0] + 4
                return fn

            # output projection in two passes accumulated in SBUF: heads 0-3
            # right after norm(3) (mid-schedule filler), heads 4-7 at the tail.
            ysum_t = big.tile([P, 8, 512], f32, name="ysum")
            op_units = []
            op_start = [10 ** 9]

            def mk_op_pass1(nbk):
                def u():
                    pg = pgen.tile([P, 512], f32, name="pg")
                    MM(pg[:], ones_s[0:1, 0:P], bias_s, start=True, stop=False)
                    for hp in range(3):
                        MM(pg[:], scp[hp][:, nbk * P:(nbk + 1) * P],
                           wpt_s[:, hp * C:(hp + 1) * C],
                           start=False, stop=(hp == 2))
                    st["pe"] += 4 * MMT
                    nc.vector.tensor_copy(ysum_t[:, nbk, :], pg[:])
                return u

            def queue_op_partials():
                for nbk in range(8):
                    op_units.append(mk_op_pass1(nbk))

            # minimum filler pace (bundle idx by step) to avoid deadline bursts
            pace_pts = [(0, 4), (16, 15), (26, 20), (40, 25), (58, 33),
                        (88, 40), (96, 41)]

            def min_b(s):
                for (s0, i0), (s1, i1) in zip(pace_pts, pace_pts[1:]):
                    if s <= s1:
                        return i0 + (i1 - i0) * (s - s0) // max(s1 - s0, 1)
                return pace_pts[-1][1]

            FORCE_LAG = 10
            LAGP = 3
            NPAIRS_ALL = H * NCHUNK
            sA = 0  # next attn@v chunk index (global, trails the S stream)

            def a_edep(i):
                return i  # S/E step producing this chunk

            def emit_a_next():
                nonlocal sA
                h2, c2 = divmod(sA, NCHUNK)
                consume_until(req_a(h2, c2))
                emit_a(h2, c2)
                if c2 == NCHUNK - 1:
                    post.append((cur_s[0] + 1, mk_sccopy(h2)))
                    post.append((cur_s[0] + 3, mk_dennorm(h2)))
                sA += 1

            cur_s = [0]
            for s in range(STEPS + LAG):
                cur_s[0] = s
                for due, fn in [pf for pf in post if pf[0] <= s]:
                    fn()
                    post.remove((due, fn))
                if s < STEPS:
                    h, c = divmod(s, NCHUNK)
                    consume_until(req_s(h, c))
                    emit_s(h, c)
                    emit_e(h, c)
                # forced attn@v to respect the ex-buffer window; tighten the
                # lag near the end so the epilogue starts promptly
                flag = FORCE_LAG if s < 112 else 3
                while sA < NPAIRS_ALL and a_edep(sA) <= s - flag:
                    emit_a_next()
                # forced output-projection pass-1: reserved filler for the
                # final-head stretch where proj/v bundles are exhausted
                if op_units and s >= op_start[0] and s % 4 == 0:
                    op_units.pop(0)()
                # greedy: fill PE up to the ACT virtual clock
                while st["pe"] < st["act"] - 0.1:
                    if st["b"] < min_b(s) and filler_units:
                        consume_unit()
                    elif sA < NPAIRS_ALL and a_edep(sA) <= s - LAGP:
                        emit_a_next()
                    elif filler_units:
                        consume_unit()
                    else:
                        break

            # drain attn@v, filler, rope pipeline, epilogues
            while sA < NPAIRS_ALL:
                emit_a_next()
            while filler_units:
                consume_unit()
            while rope_fifo:
                st["b"] += 1
                advance_rope()
            for due, fn in sorted(post, key=lambda pf: pf[0]):
                fn()
            post.clear()
            while op_units:
                op_units.pop(0)()

            # ---------------- output projection tail (heads 6-7) ----------
            for nbk in range(8):
                pg = pgen.tile([P, 512], f32, name="pg")
                MM(pg[:], scp[3][:, nbk * P:(nbk + 1) * P],
                   wpt_s[:, 3 * C:4 * C], start=True, stop=True)
                y_s = ypool.tile([P, C], f32, name="y_s")
                nc.vector.tensor_add(y_s[:], ysum_t[:, nbk, :], pg[:])
                nc.sync.dma_start(y_d.ap()[nbk * P:(nbk + 1) * P, 0:256],
                                  y_s[:, 0:256])
                nc.sync.dma_start(y_d.ap()[nbk * P:(nbk + 1) * P, 256:512],
                                  y_s[:, 256:512])

    nc.compile()
    return nc


def _get_module():
    if "nc" not in _CACHE:
        _CACHE["nc"] = _build_module()
    return _CACHE["nc"]


def kernel(x, mask, times, Wqkv, Wproj, bproj, num_cls_token=0, _trace=False):
    from concourse.bass_utils import run_bass_kernel_spmd

    assert int(num_cls_token) == 0, "kernel specialized for num_cls_token=0"
    in_maps = _host_prep(x, mask, times, Wqkv, Wproj, bproj)
    nc = _get_module()
    res = run_bass_kernel_spmd(nc, in_maps, list(range(8)), trace=_trace)
    _CACHE["last_result"] = res

    out = np.empty((B, N, C), np.float32)
    for core in range(8):
        b, qhalf = core // 2, core % 2
        out[b, qhalf * NQ:(qhalf + 1) * NQ, :] = res.results[core]["y"]
    return out


# revision 70
# speedup vs baseline: 1.0022x; 1.0022x over previous
"""Trainium2 Bass kernel for AttentionWithRotaryPositionalEmbedding.

Problem shapes (hardcoded): x [4, 2048, 512], 8 heads, head dim 64.
Sharding: 8 cores = (batch b = core//2) x (query half = core%2).
Each core computes a [1024, 512] slice of the output; k/v are computed
locally from the full x[b] so no collectives are needed.

Key perf insight (measured on hw): the PE runs matmuls at 2.4 GHz only
while it has a continuous backlog of ready work; whenever it idles
waiting on semaphores (e.g. softmax EXP on the ACT engine pacing the
attention loop), its clock drops to 1.2 GHz and every matmul takes 2x.
So this kernel emits ONE globally software-pipelined schedule: the
scores->EXP->attn@v chunk pipeline is interleaved with "filler" work
(qkv projection chains, RoPE rotation matmuls, v-projection chains,
per-head softmax-denominator matmuls) consumed greedily whenever the
estimated PE timeline falls behind the estimated ACT timeline.

Attention math per core (fp16 operands, f32 psum accumulation):
  scores sT[k,q] = kT_h^T qT_h per 128-key chunk; EXP on ACT with fused
  *0.125; attn@v with lhsT = [v_h | exp(mask)] (M=65) accumulated over
  16 chunks (psum row 64 = softmax denominators); per-head denominator
  replication via K=1 ones matmul + fast reciprocal + normalize; output
  projection as a dense tail of accumulation chains with bias folded in
  as a K=1 matmul.
"""

import sys

import numpy as np

if "/opt/trn_rl_repo" not in sys.path:
    sys.path.insert(0, "/opt/trn_rl_repo")

B, N, C = 4, 2048, 512
H, DH = 8, 64
NQ = 1024  # queries per core
P = 128
NCHUNK = N // P  # 16 k chunks
VW = DH + 1  # v columns incl. the emask/ones column
MAX_FPS = np.float32(30.0)

_CACHE = {}


def _host_prep(x, mask, times, Wqkv, Wproj, bproj):
    """Build per-core input maps (numpy only)."""
    x = np.asarray(x, np.float32)
    mask = np.asarray(mask, np.float32)
    times = np.asarray(times, np.float32)
    Wqkv = np.asarray(Wqkv, np.float32)
    Wproj = np.asarray(Wproj, np.float32)
    bproj = np.asarray(bproj, np.float32).reshape(1, C)

    wt = np.ascontiguousarray(Wqkv.T).astype(np.float16)  # [512,1536]=[WqT|WkT|WvT]
    wpt = np.ascontiguousarray(Wproj.T).astype(np.float16)  # [512, 512]

    # pairwise rotation permutation: (M2 @ v)[2i] = -v[2i+1]; [2i+1] = +v[2i]
    M2 = np.zeros((P, P), np.float16)
    for i in range(P // 2):
        M2[2 * i, 2 * i + 1] = -1.0
        M2[2 * i + 1, 2 * i] = 1.0
    m2t = np.ascontiguousarray(M2.T)

    # rotary tables (computed f32 on host, stored fp16 on device)
    inv_freq = (np.float32(1.0) /
                (np.float32(10000.0) **
                 (np.arange(0, DH, 2, dtype=np.float32) / np.float32(DH))))  # [32]
    pos = np.round(times * MAX_FPS)  # [B, N] f32, round-half-even like jnp

    in_maps = []
    for core in range(8):
        b, qhalf = core // 2, core % 2
        if qhalf == 0:
            perm = np.arange(N)
        else:
            perm = np.r_[NQ:N, 0:NQ]
        xt = np.ascontiguousarray(x[b].T[:, perm]).astype(np.float16)  # [512, 2048]
        freqs = pos[b][perm][None, :] * inv_freq[:, None]     # [32, 2048] f32
        cos32 = np.cos(freqs.astype(np.float32))
        sin32 = np.sin(freqs.astype(np.float32))
        ridx = (np.arange(P) % DH) // 2                       # row -> pair index
        ce = np.concatenate([cos32[ridx], sin32[ridx]], axis=1)  # [128, 4096]
        ce = np.ascontiguousarray(ce.astype(np.float16))
        em = np.exp(mask[b][perm]).astype(np.float32)         # [2048]
        emask = np.ascontiguousarray(em.reshape(NCHUNK, P).T)  # [128, 16]
        # chalf (fp16): m2t 0:128 | ones 128:256 | bias row 256:768
        chalf = np.zeros((P, 768), np.float16)
        chalf[:, 0:128] = m2t
        chalf[:, 128:256] = 1.0
        chalf[0, 256:768] = bproj[0].astype(np.float16)
        in_maps.append({
            "xt": xt, "wt": wt, "wpt": wpt,
            "ce": ce, "chalf": chalf, "emask": emask,
        })
    return in_maps


def _build_module():
    import concourse.tile as tile
    import concourse.mybir as mybir
    from concourse import bacc

    f32 = mybir.dt.float32
    f32r = mybir.dt.float32r
    f16 = mybir.dt.float16
    f8 = mybir.dt.float8e4
    DROW = mybir.MatmulPerfMode.DoubleRow
    nc = bacc.Bacc(None, target_bir_lowering=False, debug=False)

    xt_d = nc.dram_tensor("xt", [C, N], f16, kind="ExternalInput")
    wt_d = nc.dram_tensor("wt", [C, 3 * C], f16, kind="ExternalInput")
    wpt_d = nc.dram_tensor("wpt", [C, C], f16, kind="ExternalInput")
    ce_d = nc.dram_tensor("ce", [P, 2 * N], f16, kind="ExternalInput")
    chalf_d = nc.dram_tensor("chalf", [P, 768], f16, kind="ExternalInput")
    emask_d = nc.dram_tensor("emask", [P, NCHUNK], f32, kind="ExternalInput")
    y_d = nc.dram_tensor("y", [NQ, C], f32, kind="ExternalOutput")

    EXPF = mybir.ActivationFunctionType.Exp
    COPYF = mybir.ActivationFunctionType.Copy
    MM = nc.tensor.matmul

    # virtual-time estimates (us) used only for schedule pacing
    MMT = 0.215
    EXPT = 0.95  # deliberately under actual (~1.1): overshoot the PE feed so
    # it never stalls (a stalled PE drops to 1.2 GHz and stays there)

    with tile.TileContext(nc) as tc:
        with (
            tc.tile_pool(name="consts", bufs=1) as consts,
            tc.tile_pool(name="big", bufs=1) as big,
            tc.tile_pool(name="expp", bufs=18) as expp,
            tc.tile_pool(name="prawp", bufs=3) as prawp,
            tc.tile_pool(name="tmps", bufs=3) as tmps,
            tc.tile_pool(name="rrp", bufs=2) as rrp,
            tc.tile_pool(name="ypool", bufs=2) as ypool,
            tc.tile_pool(name="pss", bufs=2, space="PSUM") as pss,
            tc.tile_pool(name="psav", bufs=1, space="PSUM") as psav,
            tc.tile_pool(name="pgen", bufs=2, space="PSUM") as pgen,
        ):
            # ---------------- persistent tiles ----------------
            blob = consts.tile([P, 768], f16, name="blob")
            emask_t = consts.tile([P, NCHUNK], f32, name="emask")
            # wpt packed as head PAIRS: rows 0:64 = even head dims, 64:128 odd
            wpt_s = consts.tile([P, (H // 2) * C], f16, name="wpt")
            wt_s = [big.tile([P, 3 * C], f16, name=f"wt{i}") for i in range(4)]
            xt_s = [big.tile([P, N], f16, name=f"xt{i}") for i in range(4)]
            ce_s = big.tile([P, 2 * N], f16, name="ce")
            qT = [big.tile([P, NQ], f16, name=f"qT{i}") for i in range(4)]
            kT = [big.tile([P, N], f16, name=f"kT{i}") for i in range(4)]
            # v: [tok, chunk, head, 64 v dims + emask col]
            v65 = big.tile([P, NCHUNK, H, VW], f16, name="v65")
            sc = [big.tile([VW, NQ], f16, name=f"sc{h}") for h in range(H)]
            # normalized sc packed as head pairs (via SBUF->SBUF DMA) so the
            # output projection runs full-square K=128 chains
            scp = [big.tile([P, NQ], f16, name=f"scp{i}") for i in range(H // 2)]

            m2t_s = blob[:, 0:128]
            ones_s = blob[:, 128:256]
            bias_s = blob[0:1, 256:768]
            emask_ap = emask_t[:, :]

            # ---------------- DMAs (priority order) ----------------
            # critical path first: wt q-ct0, ce piece0, xt nb0, wt k-ct0
            for ci in range(4):
                nc.sync.dma_start(wt_s[ci][:, 0:128], wt_d.ap()[ci * P:(ci + 1) * P, 0:128])
            nc.sync.dma_start(ce_s[:, 0:512], ce_d.ap()[:, 0:512])
            nc.sync.dma_start(ce_s[:, 2048:2560], ce_d.ap()[:, 2048:2560])
            for half in range(2):
                for ci in range(4):
                    nc.sync.dma_start(
                        xt_s[ci][:, half * 256:(half + 1) * 256],
                        xt_d.ap()[ci * P:(ci + 1) * P, half * 256:(half + 1) * 256])
            for ci in range(4):
                nc.sync.dma_start(wt_s[ci][:, 512:640], wt_d.ap()[ci * P:(ci + 1) * P, 512:640])
            # second token block (q/k ct0 nb=1 gates head 0 too)
            for ci in range(4):
                nc.sync.dma_start(xt_s[ci][:, 512:1024],
                                  xt_d.ap()[ci * P:(ci + 1) * P, 512:1024])
            nc.sync.dma_start(ce_s[:, 512:1024], ce_d.ap()[:, 512:1024])
            nc.sync.dma_start(ce_s[:, 2560:3072], ce_d.ap()[:, 2560:3072])
            nc.sync.dma_start(blob[:], chalf_d.ap())
            nc.sync.dma_start(emask_t[:], emask_d.ap())
            # v weights
            for ci in range(4):
                nc.sync.dma_start(wt_s[ci][:, 1024:1536], wt_d.ap()[ci * P:(ci + 1) * P, 1024:1536])
            # remaining xt
            for nb in range(2, 4):
                for ci in range(4):
                    nc.sync.dma_start(xt_s[ci][:, nb * 512:(nb + 1) * 512],
                                      xt_d.ap()[ci * P:(ci + 1) * P, nb * 512:(nb + 1) * 512])
            # remaining ce
            for piece in (slice(1024, 2048), slice(3072, 4096)):
                nc.sync.dma_start(ce_s[:, piece], ce_d.ap()[:, piece])
            # remaining wt (q ct1-3, k ct1-3)
            for ci in range(4):
                nc.sync.dma_start(wt_s[ci][:, 128:512], wt_d.ap()[ci * P:(ci + 1) * P, 128:512])
                nc.sync.dma_start(wt_s[ci][:, 640:1024], wt_d.ap()[ci * P:(ci + 1) * P, 640:1024])
            # output projection weights (needed only late)
            for hp in range(H // 2):
                nc.sync.dma_start(wpt_s[:, hp * C:(hp + 1) * C],
                                  wpt_d.ap()[hp * P:(hp + 1) * P, :])

            # ---------------- filler machinery ----------------
            st = {"pe": 0.0, "act": 0.0, "b": 0}  # b = bundles fully consumed
            rope_fifo = []  # pending proj groups: [push_idx, stage, sp, ct, nb, praw, tmp]

            def advance_rope():
                for item in list(rope_fifo):
                    age = st["b"] - item[0]
                    if item[1] == 0 and age >= 1:
                        sp, ct, nb, pr = item[2], item[3], item[4], item[5]
                        tok = nb * 512
                        t = tmps.tile([P, 2, 512], f16, name="tmp")
                        nc.vector.tensor_mul(t[:, 0, :], pr[:], ce_s[:, tok:tok + 512])
                        nc.vector.tensor_mul(t[:, 1, :], pr[:],
                                             ce_s[:, 2048 + tok:2048 + tok + 512])
                        item[1] = 1
                        item[6] = t
                    elif item[1] == 1 and age >= 2:
                        sp, ct, nb, t = item[2], item[3], item[4], item[6]
                        tok = nb * 512
                        pgm = pgen.tile([P, 512], f32, name="pg")
                        MM(pgm[:], m2t_s, t[:, 1, :], start=True, stop=True)
                        st["pe"] += MMT
                        dest = qT[ct] if sp == "q" else kT[ct]
                        nc.vector.tensor_add(dest[:, tok:tok + 512], t[:, 0, :], pgm[:])
                        rope_fifo.remove(item)

            # filler bundle order: ct0 first (gates head 0), v chunks early
            # (gate attn@v), later ct tiles spread through earlier heads.
            order = [("proj", "q", 0, 0), ("proj", "k", 0, 0),
                     ("proj", "q", 0, 1), ("proj", "k", 0, 1),
                     ("v", 0), ("proj", "k", 0, 2), ("v", 1),
                     ("proj", "k", 0, 3), ("v", 2), ("v", 3), ("v", 4),
                     ("proj", "q", 1, 0), ("v", 5), ("proj", "q", 1, 1),
                     ("v", 6), ("proj", "k", 1, 0), ("v", 7),
                     ("proj", "k", 1, 1), ("v", 8), ("proj", "k", 1, 2),
                     ("v", 9), ("proj", "k", 1, 3), ("v", 10), ("v", 11),
                     ("v", 12), ("v", 13),
                     ("proj", "q", 2, 0), ("v", 14), ("proj", "q", 2, 1),
                     ("v", 15),
                     ("proj", "k", 2, 0), ("proj", "k", 2, 1),
                     ("proj", "k", 2, 2), ("proj", "k", 2, 3),
                     ("proj", "q", 3, 0), ("proj", "q", 3, 1),
                     ("proj", "k", 3, 0), ("proj", "k", 3, 1),
                     ("proj", "k", 3, 2), ("proj", "k", 3, 3),
                     ("flush", 0), ("flush", 1)]
            pos = {it: i for i, it in enumerate(order)}

            # expand bundles into ~2-matmul filler units for fine interleave
            filler_units = []
            bctx = {}

            def mk_proj_units(sp, ct, nb):
                woff = 0 if sp == "q" else C
                key = (sp, ct, nb)

                def u1():
                    pg = pgen.tile([P, 512], f32, name="pg")
                    bctx[key] = pg
                    for ci in range(2):
                        MM(pg[:],
                           wt_s[ci][:, woff + ct * P: woff + (ct + 1) * P],
                           xt_s[ci][:, nb * 512:(nb + 1) * 512],
                           start=(ci == 0), stop=False)
                    st["pe"] += 2 * MMT

                def u2():
                    pg = bctx.pop(key)
                    for ci in range(2, 4):
                        MM(pg[:],
                           wt_s[ci][:, woff + ct * P: woff + (ct + 1) * P],
                           xt_s[ci][:, nb * 512:(nb + 1) * 512],
                           start=False, stop=(ci == 3))
                    st["pe"] += 2 * MMT
                    pr = prawp.tile([P, 512], f16, name="praw")
                    nc.vector.tensor_copy(pr[:], pg[:])
                    if ct == 0:
                        # preamble fast path: full RoPE chain inline (latency
                        # over throughput -- PE is DMA-stalled here anyway)
                        tok = nb * 512
                        t = tmps.tile([P, 2, 512], f16, name="tmp")
                        nc.vector.tensor_mul(t[:, 0, :], pr[:],
                                             ce_s[:, tok:tok + 512])
                        nc.vector.tensor_mul(t[:, 1, :], pr[:],
                                             ce_s[:, 2048 + tok:2048 + tok + 512])
                        pgm = pgen.tile([P, 512], f32, name="pg")
                        MM(pgm[:], m2t_s, t[:, 1, :], start=True, stop=True)
                        st["pe"] += MMT
                        dest = qT[ct] if sp == "q" else kT[ct]
                        nc.vector.tensor_add(dest[:, tok:tok + 512],
                                             t[:, 0, :], pgm[:])
                    else:
                        rope_fifo.append([st["b"], 0, sp, ct, nb, pr, None])
                    st["b"] += 1
                    advance_rope()
                return [u1, u2]

            def mk_v_units(c):
                key = ("v", c)

                def u1():
                    pg = pgen.tile([P, 512], f32, name="pg")
                    bctx[key] = pg
                    for ci in range(2):
                        MM(pg[:],
                           xt_s[ci][:, c * P:(c + 1) * P],
                           wt_s[ci][:, 2 * C:3 * C],
                           start=(ci == 0), stop=False)
                    st["pe"] += 2 * MMT

                def u2():
                    pg = bctx.pop(key)
                    for ci in range(2, 4):
                        MM(pg[:],
                           xt_s[ci][:, c * P:(c + 1) * P],
                           wt_s[ci][:, 2 * C:3 * C],
                           start=False, stop=(ci == 3))
                    st["pe"] += 2 * MMT
                    vv = v65[:, c, :, :]
                    # evacuate on ACT (idle during lead-in; DVE is the
                    # early-phase bottleneck)
                    nc.scalar.activation(
                        vv[:, :, 0:DH],
                        pg[:].rearrange("p (h w) -> p h w", w=DH),
                        COPYF, scale=emask_ap[:, c:c + 1])
                    st["act"] = max(st["act"], st["pe"]) + 0.78
                    nc.vector.tensor_copy(
                        vv[:, :, DH:DH + 1],
                        emask_ap[:, c:c + 1, None].to_broadcast((P, H, 1)))
                    st["b"] += 1
                    advance_rope()
                return [u1, u2]

            def mk_flush_unit():
                def u():
                    st["b"] += 1
                    advance_rope()
                return [u]

            for it in order:
                if it[0] == "proj":
                    filler_units.extend(mk_proj_units(it[1], it[2], it[3]))
                elif it[0] == "v":
                    filler_units.extend(mk_v_units(it[1]))
                else:
                    filler_units.extend(mk_flush_unit())

            def consume_unit():
                filler_units.pop(0)()

            def consume_until(idx):
                while st["b"] <= min(idx, len(order) - 1) and filler_units:
                    consume_unit()

            # bundle index that must be consumed before S(h, c) / A(h, c)
            def req_s(h, c):
                ct = h // 2
                lag = 0 if ct == 0 else 2  # ct0 ropes inline, no pipeline lag
                return max(pos[("proj", "q", ct, 0)], pos[("proj", "q", ct, 1)],
                           pos[("proj", "k", ct, c // 4)]) + lag
            def req_a(h, c):
                return pos[("v", c)] + 1

            # ---------------- attention pipeline ----------------
            LAG = 3
            STEPS = H * NCHUNK
            exs = {}
            pss_live = {}
            psos = {}
            post = []  # (due_step, fn) queue for per-head epilogue work

            def emit_s(h, c):
                kt, qt, pb = kT[h // 2], qT[h // 2], (h % 2) * DH
                ps = pss.tile([P, NQ], f32, name="ps_s")
                for qb in range(2):
                    MM(ps[:, qb * 512:(qb + 1) * 512],
                       kt[pb:pb + DH, c * P:(c + 1) * P],
                       qt[pb:pb + DH, qb * 512:(qb + 1) * 512],
                       start=True, stop=True)
                st["pe"] += 2 * MMT
                pss_live[(h, c)] = ps

            def emit_e(h, c):
                ps = pss_live.pop((h, c))
                ex = expp.tile([P, NQ], f16, name="ex")
                nc.scalar.activation(ex[:], ps[:], EXPF, scale=0.125)
                st["act"] = max(st["act"], st["pe"]) + EXPT
                exs[(h, c)] = ex

            def emit_a(h, c):
                if c == 0:
                    psos[h] = psav.tile([VW, NQ], f32, name="ps_o")
                ps_o = psos[h]
                ex = exs.pop((h, c))
                for qb in range(2):
                    MM(ps_o[:, qb * 512:(qb + 1) * 512],
                       v65[:, c, h, :],
                       ex[:, qb * 512:(qb + 1) * 512],
                       start=(c == 0), stop=(c == NCHUNK - 1))
                st["pe"] += 2 * MMT

            def mk_sccopy(h):
                def fn():
                    nc.vector.tensor_copy(sc[h][:], psos.pop(h)[:])
                return fn

            def mk_dennorm(h):
                def fn():
                    psd = pss.tile([P, NQ], f32, name="ps_s")
                    for qb in range(2):
                        MM(psd[0:DH, qb * 512:(qb + 1) * 512],
                           ones_s[DH:DH + 1, 0:DH],
                           sc[h][DH:DH + 1, qb * 512:(qb + 1) * 512],
                           start=True, stop=True)
                    st["pe"] += 2 * MMT
                    rr = rrp.tile([DH, NQ], f32, name="rr")
                    nc.vector.reciprocal_approx_fast(rr[:], psd[0:DH, :])
                    nc.vector.tensor_mul(sc[h][0:DH, :], sc[h][0:DH, :], rr[:])
                    for piece in range(2):
                        nc.sync.dma_start(
                            scp[h // 2][(h % 2) * DH:(h % 2) * DH + DH,
                                        piece * 512:(piece + 1) * 512],
                            sc[h][0:DH, piece * 512:(piece + 1) * 512])
                    if h == 5:
                        queue_op_partials()
                        op_start[0] = cur_s[0] + 4
                return fn

            # output projection in two passes accumulated in SBUF: heads 0-3
            # right after norm(3) (mid-schedule filler), heads 4-7 at the tail.
            ysum_t = big.tile([P, 8, 512], f32, name="ysum")
            op_units = []
            op_start = [10 ** 9]

            def mk_op_pass1(nbk):
                def u():
                    pg = pgen.tile([P, 512], f32, name="pg")
                    MM(pg[:], ones_s[0:1, 0:P], bias_s, start=True, stop=False)
                    for hp in range(3):
                        MM(pg[:], scp[hp][:, nbk * P:(nbk + 1) * P],
                           wpt_s[:, hp * C:(hp + 1) * C],
                           start=False, stop=(hp == 2))
                    st["pe"] += 4 * MMT
                    nc.vector.tensor_copy(ysum_t[:, nbk, :], pg[:])
                return u

            def queue_op_partials():
                for nbk in range(8):
                    op_units.append(mk_op_pass1(nbk))

            # minimum filler pace (bundle idx by step) to avoid deadline bursts
            pace_pts = [(0, 4), (16, 15), (26, 20), (40, 25), (58, 33),
                        (88, 40), (96, 41)]

            def min_b(s):
                for (s0, i0), (s1, i1) in zip(pace_pts, pace_pts[1:]):
                    if s <= s1:
                        return i0 + (i1 - i0) * (s - s0) // max(s1 - s0, 1)
                return pace_pts[-1][1]

            FORCE_LAG = 10
            LAGP = 3
            NPAIRS_ALL = H * NCHUNK
            sA = 0  # next attn@v chunk index (global, trails the S stream)

            def a_edep(i):
                return i  # S/E step producing this chunk

            def emit_a_next():
                nonlocal sA
                h2, c2 = divmod(sA, NCHUNK)
                consume_until(req_a(h2, c2))
                emit_a(h2, c2)
                if c2 == NCHUNK - 1:
                    post.append((cur_s[0] + 1, mk_sccopy(h2)))
                    post.append((cur_s[0] + 3, mk_dennorm(h2)))
                sA += 1

            cur_s = [0]
            for s in range(STEPS + LAG):
                cur_s[0] = s
                for due, fn in [pf for pf in post if pf[0] <= s]:
                    fn()
                    post.remove((due, fn))
                if s < STEPS:
                    h, c = divmod(s, NCHUNK)
                    consume_until(req_s(h, c))
                    emit_s(h, c)
                    emit_e(h, c)
                # forced attn@v to respect the ex-buffer window; tighten the
                # lag near the end so the epilogue starts promptly
                flag = FORCE_LAG if s < 112 else max(3, FORCE_LAG - (s - 112))
                while sA < NPAIRS_ALL and a_edep(sA) <= s - flag:
                    emit_a_next()
                # forced output-projection pass-1: reserved filler for the
                # final-head stretch where proj/v bundles are exhausted
                if op_units and s >= op_start[0] and s % 4 == 0:
                    op_units.pop(0)()
                # greedy: fill PE up to the ACT virtual clock
                while st["pe"] < st["act"] - 0.1:
                    if st["b"] < min_b(s) and filler_units:
                        consume_unit()
                    elif sA < NPAIRS_ALL and a_edep(sA) <= s - LAGP:
                        emit_a_next()
                    elif filler_units:
                        consume_unit()
                    else:
                        break

            # drain attn@v, filler, rope pipeline, epilogues
            while sA < NPAIRS_ALL:
                emit_a_next()
            while filler_units:
                consume_unit()
            while rope_fifo:
                st["b"] += 1
                advance_rope()
            for due, fn in sorted(post, key=lambda pf: pf[0]):
                fn()
            post.clear()
            while op_units:
                op_units.pop(0)()

            # ---------------- output projection tail (heads 6-7) ----------
            for nbk in range(8):
                pg = pgen.tile([P, 512], f32, name="pg")
                MM(pg[:], scp[3][:, nbk * P:(nbk + 1) * P],
                   wpt_s[:, 3 * C:4 * C], start=True, stop=True)
                y_s = ypool.tile([P, C], f32, name="y_s")
                nc.vector.tensor_add(y_s[:], ysum_t[:, nbk, :], pg[:])
                nc.sync.dma_start(y_d.ap()[nbk * P:(nbk + 1) * P, 0:256],
                                  y_s[:, 0:256])
                nc.sync.dma_start(y_d.ap()[nbk * P:(nbk + 1) * P, 256:512],
                                  y_s[:, 256:512])

    nc.compile()
    return nc


def _get_module():
    if "nc" not in _CACHE:
        _CACHE["nc"] = _build_module()
    return _CACHE["nc"]


def kernel(x, mask, times, Wqkv, Wproj, bproj, num_cls_token=0, _trace=False):
    from concourse.bass_utils import run_bass_kernel_spmd

    assert int(num_cls_token) == 0, "kernel specialized for num_cls_token=0"
    in_maps = _host_prep(x, mask, times, Wqkv, Wproj, bproj)
    nc = _get_module()
    res = run_bass_kernel_spmd(nc, in_maps, list(range(8)), trace=_trace)
    _CACHE["last_result"] = res

    out = np.empty((B, N, C), np.float32)
    for core in range(8):
        b, qhalf = core // 2, core % 2
        out[b, qhalf * NQ:(qhalf + 1) * NQ, :] = res.results[core]["y"]
    return out


# revision 75
# speedup vs baseline: 1.2099x; 1.2072x over previous
"""Trainium2 Bass kernel for AttentionWithRotaryPositionalEmbedding.

Problem shapes (hardcoded): x [4, 2048, 512], 8 heads, head dim 64.
Sharding: 8 cores = (batch b = core//2) x (query half = core%2).
Each core computes a [1024, 512] slice of the output; k/v are computed
locally from the full x[b] so no collectives are needed.

Key perf insight (measured on hw): the PE runs matmuls at 2.4 GHz only
while it has a continuous backlog of ready work; whenever it idles
waiting on semaphores (e.g. softmax EXP on the ACT engine pacing the
attention loop), its clock drops to 1.2 GHz and every matmul takes 2x.
So this kernel emits ONE globally software-pipelined schedule: the
scores->EXP->attn@v chunk pipeline is interleaved with "filler" work
(qkv projection chains, RoPE rotation matmuls, v-projection chains,
per-head softmax-denominator matmuls) consumed greedily whenever the
estimated PE timeline falls behind the estimated ACT timeline.

Attention math per core (fp16 operands, f32 psum accumulation):
  scores sT[k,q] = kT_h^T qT_h per 128-key chunk; EXP on ACT with fused
  *0.125; attn@v with lhsT = [v_h | exp(mask)] (M=65) accumulated over
  16 chunks (psum row 64 = softmax denominators); per-head denominator
  replication via K=1 ones matmul + fast reciprocal + normalize; output
  projection as a dense tail of accumulation chains with bias folded in
  as a K=1 matmul.
"""

import sys

import numpy as np

if "/opt/trn_rl_repo" not in sys.path:
    sys.path.insert(0, "/opt/trn_rl_repo")

B, N, C = 4, 2048, 512
H, DH = 8, 64
NQ = 1024  # queries per core
P = 128
NCHUNK = N // P  # 16 k chunks
VW = DH + 1  # v columns incl. the emask/ones column
MAX_FPS = np.float32(30.0)

_CACHE = {}


def _host_prep(x, mask, times, Wqkv, Wproj, bproj):
    """Build per-core input maps (numpy only)."""
    x = np.asarray(x, np.float32)
    mask = np.asarray(mask, np.float32)
    times = np.asarray(times, np.float32)
    Wqkv = np.asarray(Wqkv, np.float32)
    Wproj = np.asarray(Wproj, np.float32)
    bproj = np.asarray(bproj, np.float32).reshape(1, C)

    wt = np.ascontiguousarray(Wqkv.T).astype(np.float16)  # [512,1536]=[WqT|WkT|WvT]
    wpt = np.ascontiguousarray(Wproj.T).astype(np.float16)  # [512, 512]

    # pairwise rotation permutation: (M2 @ v)[2i] = -v[2i+1]; [2i+1] = +v[2i]
    M2 = np.zeros((P, P), np.float16)
    for i in range(P // 2):
        M2[2 * i, 2 * i + 1] = -1.0
        M2[2 * i + 1, 2 * i] = 1.0
    m2t = np.ascontiguousarray(M2.T)

    # rotary tables (computed f32 on host, stored fp16 on device)
    inv_freq = (np.float32(1.0) /
                (np.float32(10000.0) **
                 (np.arange(0, DH, 2, dtype=np.float32) / np.float32(DH))))  # [32]
    pos = np.round(times * MAX_FPS)  # [B, N] f32, round-half-even like jnp

    in_maps = []
    for core in range(8):
        b, qhalf = core // 2, core % 2
        if qhalf == 0:
            perm = np.arange(N)
        else:
            perm = np.r_[NQ:N, 0:NQ]
        xt = np.ascontiguousarray(x[b].T[:, perm]).astype(np.float16)  # [512, 2048]
        freqs = pos[b][perm][None, :] * inv_freq[:, None]     # [32, 2048] f32
        cos32 = np.cos(freqs.astype(np.float32))
        sin32 = np.sin(freqs.astype(np.float32))
        ridx = (np.arange(P) % DH) // 2                       # row -> pair index
        ce = np.concatenate([cos32[ridx], sin32[ridx]], axis=1)  # [128, 4096]
        ce = np.ascontiguousarray(ce.astype(np.float16))
        em = np.exp(mask[b][perm]).astype(np.float32)         # [2048]
        emask = np.ascontiguousarray(em.reshape(NCHUNK, P).T)  # [128, 16]
        # chalf (fp16): m2t 0:128 | ones 128:256 | bias row 256:768
        chalf = np.zeros((P, 768), np.float16)
        chalf[:, 0:128] = m2t
        chalf[:, 128:256] = 1.0
        chalf[0, 256:768] = bproj[0].astype(np.float16)
        in_maps.append({
            "xt": xt, "wt": wt, "wpt": wpt,
            "ce": ce, "chalf": chalf, "emask": emask,
        })
    return in_maps


def _build_module():
    import concourse.tile as tile
    import concourse.mybir as mybir
    from concourse import bacc

    f32 = mybir.dt.float32
    f32r = mybir.dt.float32r
    f16 = mybir.dt.float16
    f8 = mybir.dt.float8e4
    DROW = mybir.MatmulPerfMode.DoubleRow
    nc = bacc.Bacc(None, target_bir_lowering=False, debug=False)

    xt_d = nc.dram_tensor("xt", [C, N], f16, kind="ExternalInput")
    wt_d = nc.dram_tensor("wt", [C, 3 * C], f16, kind="ExternalInput")
    wpt_d = nc.dram_tensor("wpt", [C, C], f16, kind="ExternalInput")
    ce_d = nc.dram_tensor("ce", [P, 2 * N], f16, kind="ExternalInput")
    chalf_d = nc.dram_tensor("chalf", [P, 768], f16, kind="ExternalInput")
    emask_d = nc.dram_tensor("emask", [P, NCHUNK], f32, kind="ExternalInput")
    y_d = nc.dram_tensor("y", [NQ, C], f32, kind="ExternalOutput")

    EXPF = mybir.ActivationFunctionType.Exp
    COPYF = mybir.ActivationFunctionType.Copy
    MM = nc.tensor.matmul

    # virtual-time estimates (us) used only for schedule pacing
    MMT = 0.215
    EXPT = 0.95  # deliberately under actual (~1.1): overshoot the PE feed so
    # it never stalls (a stalled PE drops to 1.2 GHz and stays there)

    with tile.TileContext(nc) as tc:
        with (
            tc.tile_pool(name="consts", bufs=1) as consts,
            tc.tile_pool(name="big", bufs=1) as big,
            tc.tile_pool(name="expp", bufs=18) as expp,
            tc.tile_pool(name="prawp", bufs=3) as prawp,
            tc.tile_pool(name="tmps", bufs=3) as tmps,
            tc.tile_pool(name="rrp", bufs=2) as rrp,
            tc.tile_pool(name="ypool", bufs=2) as ypool,
            tc.tile_pool(name="pss", bufs=2, space="PSUM") as pss,
            tc.tile_pool(name="psav", bufs=1, space="PSUM") as psav,
            tc.tile_pool(name="pgen", bufs=2, space="PSUM") as pgen,
        ):
            # ---------------- persistent tiles ----------------
            blob = consts.tile([P, 768], f16, name="blob")
            emask_t = consts.tile([P, NCHUNK], f32, name="emask")
            # wpt packed as head PAIRS: rows 0:64 = even head dims, 64:128 odd
            wpt_s = consts.tile([P, (H // 2) * C], f16, name="wpt")
            wt_s = [big.tile([P, 3 * C], f16, name=f"wt{i}") for i in range(4)]
            xt_s = [big.tile([P, N], f16, name=f"xt{i}") for i in range(4)]
            ce_s = big.tile([P, 2 * N], f16, name="ce")
            qT = [big.tile([P, NQ], f16, name=f"qT{i}") for i in range(4)]
            kT = [big.tile([P, N], f16, name=f"kT{i}") for i in range(4)]
            # v: [tok, chunk, head, 64 v dims + emask col]
            v65 = big.tile([P, NCHUNK, H, VW], f16, name="v65")
            sc = [big.tile([VW, NQ], f16, name=f"sc{h}") for h in range(H)]
            # normalized sc packed as head pairs (via SBUF->SBUF DMA) so the
            # output projection runs full-square K=128 chains
            scp = [big.tile([P, NQ], f16, name=f"scp{i}") for i in range(H // 2)]

            m2t_s = blob[:, 0:128]
            ones_s = blob[:, 128:256]
            bias_s = blob[0:1, 256:768]
            emask_ap = emask_t[:, :]

            # ---------------- DMAs (priority order) ----------------
            # critical path first: wt q-ct0, ce piece0, xt nb0, wt k-ct0
            for ci in range(4):
                nc.sync.dma_start(wt_s[ci][:, 0:128], wt_d.ap()[ci * P:(ci + 1) * P, 0:128])
            nc.sync.dma_start(ce_s[:, 0:512], ce_d.ap()[:, 0:512])
            nc.sync.dma_start(ce_s[:, 2048:2560], ce_d.ap()[:, 2048:2560])
            nc.sync.dma_start(blob[:], chalf_d.ap())
            nc.sync.dma_start(emask_t[:], emask_d.ap())
            for half in range(2):
                for ci in range(4):
                    nc.sync.dma_start(
                        xt_s[ci][:, half * 256:(half + 1) * 256],
                        xt_d.ap()[ci * P:(ci + 1) * P, half * 256:(half + 1) * 256])
            for ci in range(4):
                nc.sync.dma_start(wt_s[ci][:, 512:640], wt_d.ap()[ci * P:(ci + 1) * P, 512:640])
            # second token block (q/k ct0 nb=1 gates head 0 too)
            for ci in range(4):
                nc.sync.dma_start(xt_s[ci][:, 512:1024],
                                  xt_d.ap()[ci * P:(ci + 1) * P, 512:1024])
            nc.sync.dma_start(ce_s[:, 512:1024], ce_d.ap()[:, 512:1024])
            nc.sync.dma_start(ce_s[:, 2560:3072], ce_d.ap()[:, 2560:3072])
            nc.sync.dma_start(blob[:], chalf_d.ap())
            nc.sync.dma_start(emask_t[:], emask_d.ap())
            # v weights
            for ci in range(4):
                nc.sync.dma_start(wt_s[ci][:, 1024:1536], wt_d.ap()[ci * P:(ci + 1) * P, 1024:1536])
            # remaining xt
            for nb in range(2, 4):
                for ci in range(4):
                    nc.sync.dma_start(xt_s[ci][:, nb * 512:(nb + 1) * 512],
                                      xt_d.ap()[ci * P:(ci + 1) * P, nb * 512:(nb + 1) * 512])
            # remaining ce
            for piece in (slice(1024, 2048), slice(3072, 4096)):
                nc.sync.dma_start(ce_s[:, piece], ce_d.ap()[:, piece])
            # remaining wt (q ct1-3, k ct1-3)
            for ci in range(4):
                nc.sync.dma_start(wt_s[ci][:, 128:512], wt_d.ap()[ci * P:(ci + 1) * P, 128:512])
                nc.sync.dma_start(wt_s[ci][:, 640:1024], wt_d.ap()[ci * P:(ci + 1) * P, 640:1024])
            # output projection weights (needed only late)
            for hp in range(H // 2):
                nc.sync.dma_start(wpt_s[:, hp * C:(hp + 1) * C],
                                  wpt_d.ap()[hp * P:(hp + 1) * P, :])

            # ---------------- filler machinery ----------------
            st = {"pe": 0.0, "act": 0.0, "b": 0}  # b = bundles fully consumed
            rope_fifo = []  # pending proj groups: [push_idx, stage, sp, ct, nb, praw, tmp]

            def advance_rope():
                for item in list(rope_fifo):
                    age = st["b"] - item[0]
                    if item[1] == 0 and age >= 1:
                        sp, ct, nb, pr = item[2], item[3], item[4], item[5]
                        tok = nb * 512
                        t = tmps.tile([P, 2, 512], f16, name="tmp")
                        nc.vector.tensor_mul(t[:, 0, :], pr[:], ce_s[:, tok:tok + 512])
                        nc.vector.tensor_mul(t[:, 1, :], pr[:],
                                             ce_s[:, 2048 + tok:2048 + tok + 512])
                        item[1] = 1
                        item[6] = t
                    elif item[1] == 1 and age >= 2:
                        sp, ct, nb, t = item[2], item[3], item[4], item[6]
                        tok = nb * 512
                        pgm = pgen.tile([P, 512], f32, name="pg")
                        MM(pgm[:], m2t_s, t[:, 1, :], start=True, stop=True)
                        st["pe"] += MMT
                        dest = qT[ct] if sp == "q" else kT[ct]
                        nc.vector.tensor_add(dest[:, tok:tok + 512], t[:, 0, :], pgm[:])
                        rope_fifo.remove(item)

            # filler bundle order: ct0 first (gates head 0), v chunks early
            # (gate attn@v), later ct tiles spread through earlier heads.
            order = [("proj", "q", 0, 0), ("proj", "k", 0, 0),
                     ("proj", "q", 0, 1), ("proj", "k", 0, 1),
                     ("v", 0), ("proj", "k", 0, 2), ("v", 1),
                     ("proj", "k", 0, 3), ("v", 2), ("v", 3), ("v", 4),
                     ("proj", "q", 1, 0), ("v", 5), ("proj", "q", 1, 1),
                     ("v", 6), ("proj", "k", 1, 0), ("v", 7),
                     ("proj", "k", 1, 1), ("v", 8), ("proj", "k", 1, 2),
                     ("v", 9), ("proj", "k", 1, 3), ("v", 10), ("v", 11),
                     ("v", 12), ("v", 13),
                     ("proj", "q", 2, 0), ("v", 14), ("proj", "q", 2, 1),
                     ("v", 15),
                     ("proj", "k", 2, 0), ("proj", "k", 2, 1),
                     ("proj", "k", 2, 2), ("proj", "k", 2, 3),
                     ("proj", "q", 3, 0), ("proj", "q", 3, 1),
                     ("proj", "k", 3, 0), ("proj", "k", 3, 1),
                     ("proj", "k", 3, 2), ("proj", "k", 3, 3),
                     ("flush", 0), ("flush", 1)]
            pos = {it: i for i, it in enumerate(order)}

            # expand bundles into ~2-matmul filler units for fine interleave
            filler_units = []
            bctx = {}

            def mk_proj_units(sp, ct, nb):
                woff = 0 if sp == "q" else C
                key = (sp, ct, nb)

                def u1():
                    pg = pgen.tile([P, 512], f32, name="pg")
                    bctx[key] = pg
                    for ci in range(2):
                        MM(pg[:],
                           wt_s[ci][:, woff + ct * P: woff + (ct + 1) * P],
                           xt_s[ci][:, nb * 512:(nb + 1) * 512],
                           start=(ci == 0), stop=False)
                    st["pe"] += 2 * MMT

                def u2():
                    pg = bctx.pop(key)
                    for ci in range(2, 4):
                        MM(pg[:],
                           wt_s[ci][:, woff + ct * P: woff + (ct + 1) * P],
                           xt_s[ci][:, nb * 512:(nb + 1) * 512],
                           start=False, stop=(ci == 3))
                    st["pe"] += 2 * MMT
                    pr = prawp.tile([P, 512], f16, name="praw")
                    nc.vector.tensor_copy(pr[:], pg[:])
                    if ct == 0:
                        # preamble fast path: full RoPE chain inline (latency
                        # over throughput -- PE is DMA-stalled here anyway).
                        # M2 borrows a pss slot (idle pre-attention) so the
                        # 2-deep pgen rotation doesn't serialize the groups.
                        tok = nb * 512
                        t = tmps.tile([P, 2, 512], f16, name="tmp")
                        nc.vector.tensor_mul(t[:, 0, :], pr[:],
                                             ce_s[:, tok:tok + 512])
                        nc.vector.tensor_mul(t[:, 1, :], pr[:],
                                             ce_s[:, 2048 + tok:2048 + tok + 512])
                        pgm = pss.tile([P, NQ], f32, name="ps_s")
                        MM(pgm[:, 0:512], m2t_s, t[:, 1, :], start=True, stop=True)
                        st["pe"] += MMT
                        dest = qT[ct] if sp == "q" else kT[ct]
                        nc.vector.tensor_add(dest[:, tok:tok + 512],
                                             t[:, 0, :], pgm[:, 0:512])
                    else:
                        rope_fifo.append([st["b"], 0, sp, ct, nb, pr, None])
                    st["b"] += 1
                    advance_rope()
                return [u1, u2]

            def mk_v_units(c):
                key = ("v", c)

                def u1():
                    pg = pgen.tile([P, 512], f32, name="pg")
                    bctx[key] = pg
                    for ci in range(2):
                        MM(pg[:],
                           xt_s[ci][:, c * P:(c + 1) * P],
                           wt_s[ci][:, 2 * C:3 * C],
                           start=(ci == 0), stop=False)
                    st["pe"] += 2 * MMT

                def u2():
                    pg = bctx.pop(key)
                    for ci in range(2, 4):
                        MM(pg[:],
                           xt_s[ci][:, c * P:(c + 1) * P],
                           wt_s[ci][:, 2 * C:3 * C],
                           start=False, stop=(ci == 3))
                    st["pe"] += 2 * MMT
                    vv = v65[:, c, :, :]
                    # evacuate on ACT (idle during lead-in; DVE is the
                    # early-phase bottleneck)
                    nc.scalar.activation(
                        vv[:, :, 0:DH],
                        pg[:].rearrange("p (h w) -> p h w", w=DH),
                        COPYF, scale=emask_ap[:, c:c + 1])
                    st["act"] = max(st["act"], st["pe"]) + 0.78
                    nc.vector.tensor_copy(
                        vv[:, :, DH:DH + 1],
                        emask_ap[:, c:c + 1, None].to_broadcast((P, H, 1)))
                    st["b"] += 1
                    advance_rope()
                return [u1, u2]

            def mk_flush_unit():
                def u():
                    st["b"] += 1
                    advance_rope()
                return [u]

            for it in order:
                if it[0] == "proj":
                    filler_units.extend(mk_proj_units(it[1], it[2], it[3]))
                elif it[0] == "v":
                    filler_units.extend(mk_v_units(it[1]))
                else:
                    filler_units.extend(mk_flush_unit())

            def consume_unit():
                filler_units.pop(0)()

            def consume_until(idx):
                while st["b"] <= min(idx, len(order) - 1) and filler_units:
                    consume_unit()

            # bundle index that must be consumed before S(h, c) / A(h, c)
            def req_s(h, c):
                ct = h // 2
                lag = 0 if ct == 0 else 2  # ct0 ropes inline, no pipeline lag
                return max(pos[("proj", "q", ct, 0)], pos[("proj", "q", ct, 1)],
                           pos[("proj", "k", ct, c // 4)]) + lag
            def req_a(h, c):
                return pos[("v", c)] + 1

            # ---------------- attention pipeline ----------------
            LAG = 3
            STEPS = H * NCHUNK
            exs = {}
            pss_live = {}
            psos = {}
            post = []  # (due_step, fn) queue for per-head epilogue work

            def emit_s(h, c):
                kt, qt, pb = kT[h // 2], qT[h // 2], (h % 2) * DH
                ps = pss.tile([P, NQ], f32, name="ps_s")
                for qb in range(2):
                    MM(ps[:, qb * 512:(qb + 1) * 512],
                       kt[pb:pb + DH, c * P:(c + 1) * P],
                       qt[pb:pb + DH, qb * 512:(qb + 1) * 512],
                       start=True, stop=True)
                st["pe"] += 2 * MMT
                pss_live[(h, c)] = ps

            def emit_e(h, c):
                ps = pss_live.pop((h, c))
                ex = expp.tile([P, NQ], f16, name="ex")
                nc.scalar.activation(ex[:], ps[:], EXPF, scale=0.125)
                st["act"] = max(st["act"], st["pe"]) + EXPT
                exs[(h, c)] = ex

            def emit_a(h, c):
                if c == 0:
                    psos[h] = psav.tile([VW, NQ], f32, name="ps_o")
                ps_o = psos[h]
                ex = exs.pop((h, c))
                for qb in range(2):
                    MM(ps_o[:, qb * 512:(qb + 1) * 512],
                       v65[:, c, h, :],
                       ex[:, qb * 512:(qb + 1) * 512],
                       start=(c == 0), stop=(c == NCHUNK - 1))
                st["pe"] += 2 * MMT

            def mk_sccopy(h):
                def fn():
                    nc.vector.tensor_copy(sc[h][:], psos.pop(h)[:])
                return fn

            def mk_dennorm(h):
                def fn():
                    psd = pss.tile([P, NQ], f32, name="ps_s")
                    for qb in range(2):
                        MM(psd[0:DH, qb * 512:(qb + 1) * 512],
                           ones_s[DH:DH + 1, 0:DH],
                           sc[h][DH:DH + 1, qb * 512:(qb + 1) * 512],
                           start=True, stop=True)
                    st["pe"] += 2 * MMT
                    rr = rrp.tile([DH, NQ], f32, name="rr")
                    nc.vector.reciprocal_approx_fast(rr[:], psd[0:DH, :])
                    nc.vector.tensor_mul(sc[h][0:DH, :], sc[h][0:DH, :], rr[:])
                    for piece in range(2):
                        nc.sync.dma_start(
                            scp[h // 2][(h % 2) * DH:(h % 2) * DH + DH,
                                        piece * 512:(piece + 1) * 512],
                            sc[h][0:DH, piece * 512:(piece + 1) * 512])
                    if h == 5:
                        queue_op_partials()
                        op_start[0] = cur_s[0] + 4
                return fn

            # output projection in two passes accumulated in SBUF: heads 0-3
            # right after norm(3) (mid-schedule filler), heads 4-7 at the tail.
            ysum_t = big.tile([P, 8, 512], f32, name="ysum")
            op_units = []
            op_start = [10 ** 9]

            def mk_op_pass1(nbk):
                def u():
                    pg = pgen.tile([P, 512], f32, name="pg")
                    MM(pg[:], ones_s[0:1, 0:P], bias_s, start=True, stop=False)
                    for hp in range(3):
                        MM(pg[:], scp[hp][:, nbk * P:(nbk + 1) * P],
                           wpt_s[:, hp * C:(hp + 1) * C],
                           start=False, stop=(hp == 2))
                    st["pe"] += 4 * MMT
                    nc.vector.tensor_copy(ysum_t[:, nbk, :], pg[:])
                return u

            def queue_op_partials():
                for nbk in range(8):
                    op_units.append(mk_op_pass1(nbk))

            # minimum filler pace (bundle idx by step) to avoid deadline bursts
            pace_pts = [(0, 4), (16, 15), (26, 20), (40, 25), (58, 33),
                        (88, 40), (96, 41)]

            def min_b(s):
                for (s0, i0), (s1, i1) in zip(pace_pts, pace_pts[1:]):
                    if s <= s1:
                        return i0 + (i1 - i0) * (s - s0) // max(s1 - s0, 1)
                return pace_pts[-1][1]

            FORCE_LAG = 10
            LAGP = 3
            NPAIRS_ALL = H * NCHUNK
            sA = 0  # next attn@v chunk index (global, trails the S stream)

            def a_edep(i):
                return i  # S/E step producing this chunk

            def emit_a_next():
                nonlocal sA
                h2, c2 = divmod(sA, NCHUNK)
                consume_until(req_a(h2, c2))
                emit_a(h2, c2)
                if c2 == NCHUNK - 1:
                    post.append((cur_s[0] + 1, mk_sccopy(h2)))
                    post.append((cur_s[0] + 3, mk_dennorm(h2)))
                sA += 1

            cur_s = [0]
            for s in range(STEPS + LAG):
                cur_s[0] = s
                for due, fn in [pf for pf in post if pf[0] <= s]:
                    fn()
                    post.remove((due, fn))
                if s < STEPS:
                    h, c = divmod(s, NCHUNK)
                    consume_until(req_s(h, c))
                    emit_s(h, c)
                    emit_e(h, c)
                # forced attn@v to respect the ex-buffer window; tighten the
                # lag near the end so the epilogue starts promptly
                flag = FORCE_LAG if s < 112 else max(3, FORCE_LAG - (s - 112))
                while sA < NPAIRS_ALL and a_edep(sA) <= s - flag:
                    emit_a_next()
                # forced output-projection pass-1: reserved filler for the
                # final-head stretch where proj/v bundles are exhausted
                if op_units and s >= op_start[0] and s % 4 == 0:
                    op_units.pop(0)()
                # greedy: fill PE up to the ACT virtual clock
                while st["pe"] < st["act"] - 0.1:
                    if st["b"] < min_b(s) and filler_units:
                        consume_unit()
                    elif sA < NPAIRS_ALL and a_edep(sA) <= s - LAGP:
                        emit_a_next()
                    elif filler_units:
                        consume_unit()
                    else:
                        break

            # drain attn@v, filler, rope pipeline, epilogues
            while sA < NPAIRS_ALL:
                emit_a_next()
            while filler_units:
                consume_unit()
            while rope_fifo:
                st["b"] += 1
                advance_rope()
            for due, fn in sorted(post, key=lambda pf: pf[0]):
                fn()
            post.clear()
            while op_units:
                op_units.pop(0)()

            # ---------------- output projection tail (heads 6-7) ----------
            for nbk in range(8):
                pg = pgen.tile([P, 512], f32, name="pg")
                MM(pg[:], scp[3][:, nbk * P:(nbk + 1) * P],
                   wpt_s[:, 3 * C:4 * C], start=True, stop=True)
                y_s = ypool.tile([P, C], f32, name="y_s")
                nc.vector.tensor_add(y_s[:], ysum_t[:, nbk, :], pg[:])
                nc.sync.dma_start(y_d.ap()[nbk * P:(nbk + 1) * P, 0:256],
                                  y_s[:, 0:256])
                nc.sync.dma_start(y_d.ap()[nbk * P:(nbk + 1) * P, 256:512],
                                  y_s[:, 256:512])

    nc.compile()
    return nc


def _get_module():
    if "nc" not in _CACHE:
        _CACHE["nc"] = _build_module()
    return _CACHE["nc"]


def kernel(x, mask, times, Wqkv, Wproj, bproj, num_cls_token=0, _trace=False):
    from concourse.bass_utils import run_bass_kernel_spmd

    assert int(num_cls_token) == 0, "kernel specialized for num_cls_token=0"
    in_maps = _host_prep(x, mask, times, Wqkv, Wproj, bproj)
    nc = _get_module()
    res = run_bass_kernel_spmd(nc, in_maps, list(range(8)), trace=_trace)
    _CACHE["last_result"] = res

    out = np.empty((B, N, C), np.float32)
    for core in range(8):
        b, qhalf = core // 2, core % 2
        out[b, qhalf * NQ:(qhalf + 1) * NQ, :] = res.results[core]["y"]
    return out
